# revision 5
# baseline (speedup 1.0000x reference)
"""AdaptiveQuantizer Trainium2 kernel (8 NeuronCores, data-parallel over batch).

Math (per pixel (b,h,w), over C=64 channels):
    fmin/fmax = min/max over channels
    rng  = fmax - fmin                (clamped to >= 1e-30; rng ~ 5 for randn data)
    lm1  = 2**bits - 1                (exact, via int shift trick)
    u    = lm1 / rng
    v    = rng / lm1
    w    = u*f - u*fmin               in [0, lm1]
    r    = round_half_even(w)         via fp32 add of M = 1.5*2**23
    out  = v*r + fmin

Layout: pixels in both the partition dim (128) and the free dim (256/superblock),
channels are a strided free-axis dim of one big SBUF tile -> channel reduction is
a free-axis strided tensor_reduce; per-pixel params are [128,256] tiles applied
with broadcast (step-0) APs. All elementwise work runs in-place on the big tile.
"""

import os
import sys

for _p in ("/opt/trn_rl_repo", "/root/.axon_site/_ro/trn_rl_repo"):
    if os.path.isdir(_p) and _p not in sys.path:
        sys.path.insert(0, _p)

import numpy as np

import concourse.bass as bass
import concourse.mybir as mybir
from concourse.bass_utils import run_bass_kernel_spmd
from concourse.tile import TileContext
from concourse.vector_clock import ScopedClock

# Problem shapes (hardcoded per spec)
B_FULL, C, H, W = 16, 64, 256, 256
N_CORES = 8
B_LOC = B_FULL // N_CORES  # images per core
PX = H * W                 # pixels per image
P = 128                    # SBUF partitions
WPP = 256                  # pixels per partition per superblock
SB_PX = P * WPP            # pixels per superblock
N_SB = PX // SB_PX         # superblocks per image
CCH = 8                    # channels per DMA chunk (1 MiB per dma_start)
M_MAGIC = 12582912.0       # 1.5*2**23: fp32 "+M" add == round-to-nearest-even
AL = mybir.AluOpType
F32 = mybir.dt.float32
I32 = mybir.dt.int32

# channels handled by DVE for the big elementwise passes; the rest go to GPSIMD
DVE_CH = 64
# channels of the min/max reduction accumulated on GPSIMD via tensor_tensor
GP_RED_CH = 0

_drain_patched = False


def _patch_tile_drain():
    """This container's walrus accepts only ONE sync wait per TPB_CTRL
    instruction; Tile's final drain carries one wait per ticked proc.
    Split them across multiple drains."""
    global _drain_patched
    if _drain_patched:
        return
    _drain_patched = True

    def _patched(self, tick_clock, wait_clock):
        nc = self.nc
        drain_inst = nc.sync.drain()
        wait_clock.add_sem_waits(
            drain_inst.ins, ScopedClock({None: tick_clock.global_clock})
        )
        si = drain_inst.ins.sync_info
        waits = list(si.on_wait) if (si is not None and si.on_wait) else []
        if len(waits) > 1:
            si.on_wait = waits[:1]
            for wchunk in waits[1:]:
                extra = nc.sync.drain()
                esi = extra.ins.sync_info
                if esi is None:
                    extra.ins.sync_info = mybir.SyncInfo(
                        on_wait=[wchunk], on_update=[]
                    )
                else:
                    esi.on_wait = [wchunk]
        nc.all_engine_barrier()
        assert self.sems is not None
        popped = nc._tile_sem_poison_stack.pop()
        assert popped is self._sem_poison
        nc.clear_and_free_semaphores(list(self.sems.allocated().values()))
        nc.all_engine_barrier()

    TileContext._drain_and_barrier = _patched


def _split_sync_waits(nc: bass.Bass, max_waits: int = 1) -> None:
    """This container's walrus rejects instructions carrying more than one
    sync wait. Hoist excess waits onto injected same-engine NOPs placed
    immediately before the instruction (engine program order makes this
    semantically identical)."""
    k = 0
    for bb in nc.main_func.blocks:
        insts = list(bb.instructions)
        out_list = []
        changed = False
        for inst in insts:
            si = inst.sync_info
            waits = list(si.on_wait) if (si is not None and si.on_wait) else []
            if len(waits) > max_waits:
                keep = waits[-max_waits:]
                hoist = waits[:-max_waits]
                for i in range(0, len(hoist), max_waits):
                    nop = mybir.InstNoOp(name=f"WSPL-{k}", ins=[], outs=[])
                    k += 1
                    nop.engine = inst.engine
                    nop.sync_info = mybir.SyncInfo(
                        on_wait=hoist[i : i + max_waits], on_update=[]
                    )
                    out_list.append(nop)
                si.on_wait = keep
                changed = True
            out_list.append(inst)
        if changed:
            bb.instructions.clear()
            for inst in out_list:
                bb.instructions.append(inst)


def build(reps: int = 1) -> bass.Bass:
    """Build the per-core Bass program. `reps` repeats the whole workload
    (idempotent) for wall-clock timing amortization."""
    _patch_tile_drain()
    nc = bass.Bass()
    feat = nc.declare_dram_parameter("features", [B_LOC, C, PX], F32, isOutput=False)
    bits = nc.declare_dram_parameter("bit_allocation", [B_LOC, PX], I32, isOutput=False)
    out = nc.declare_dram_parameter("out", [B_LOC, C, PX], F32, isOutput=True)

    with TileContext(nc) as tc:
        with (
            tc.tile_pool(name="fpool", bufs=2) as fpool,
            tc.tile_pool(name="ppool", bufs=2) as ppool,
        ):
            for _rep in range(reps):
                for b in range(B_LOC):
                    for s in range(N_SB):
                        px0 = s * SB_PX
                        F = fpool.tile([P, C * WPP], F32, tag="F")
                        Fv = F[:].rearrange("p (c w) -> p c w", c=C)
                        # ---- load: 1 MiB chunks, contiguous 1 KiB runs ----
                        for cc in range(0, C, CCH):
                            src = feat[b, cc : cc + CCH, px0 : px0 + SB_PX]
                            src = src.rearrange("c (p w) -> p c w", p=P)
                            nc.sync.dma_start(out=Fv[:, cc : cc + CCH, :], in_=src)
                        bt = ppool.tile([P, WPP], I32, tag="bt")
                        nc.sync.dma_start(
                            out=bt[:],
                            in_=bits[b, px0 : px0 + SB_PX].rearrange(
                                "(p w) -> p w", p=P
                            ),
                        )

                        # ---- channel min/max (free-axis reduce, c innermost) ----
                        fmax = ppool.tile([P, WPP], F32, tag="fmax")
                        fmin = ppool.tile([P, WPP], F32, tag="fmin")
                        ndc = C - GP_RED_CH  # channels reduced on DVE
                        Fr = F[:].rearrange("p (c w) -> p w c", c=C)
                        nc.vector.tensor_reduce(
                            fmax[:], Fr[:, :, :ndc], axis=mybir.AxisListType.X, op=AL.max
                        )
                        nc.vector.tensor_reduce(
                            fmin[:], Fr[:, :, :ndc], axis=mybir.AxisListType.X, op=AL.min
                        )
                        if GP_RED_CH:
                            # accumulate the tail channels on GPSIMD (w-contiguous)
                            for c in range(ndc, C):
                                nc.gpsimd.tensor_tensor(
                                    fmax[:], fmax[:], Fv[:, c, :], AL.max
                                )
                                nc.gpsimd.tensor_tensor(
                                    fmin[:], fmin[:], Fv[:, c, :], AL.min
                                )

                        # ---- per-pixel params ([P, WPP] tiles) ----
                        rng = ppool.tile([P, WPP], F32, tag="rng")
                        nc.vector.tensor_tensor(rng[:], fmax[:], fmin[:], AL.subtract)
                        nc.vector.tensor_scalar_max(rng[:], rng[:], 1e-30)
                        rinv = ppool.tile([P, WPP], F32, tag="rinv")
                        nc.vector.reciprocal(rinv[:], rng[:])
                        # lm1 = 2**bits - 1 exactly: (bits+127)<<23 bitcast f32
                        lvl_i = ppool.tile([P, WPP], I32, tag="lvl_i")
                        nc.vector.tensor_scalar_add(lvl_i[:], bt[:], 127)
                        nc.vector.tensor_scalar(
                            lvl_i[:], lvl_i[:], 23, None, AL.logical_shift_left
                        )
                        lm1 = ppool.tile([P, WPP], F32, tag="lm1")
                        nc.vector.tensor_scalar_sub(
                            lm1[:], lvl_i[:].bitcast(F32), 1.0
                        )
                        u = ppool.tile([P, WPP], F32, tag="u")
                        nc.vector.tensor_tensor(u[:], lm1[:], rinv[:], AL.mult)
                        # c2 = -u*fmin  (keep M out: c2 must stay sub-ulp accurate)
                        c2 = ppool.tile([P, WPP], F32, tag="c2")
                        nc.vector.scalar_tensor_tensor(
                            c2[:], u[:], -1.0, fmin[:], AL.mult, AL.mult
                        )
                        ilm1 = ppool.tile([P, WPP], F32, tag="ilm1")
                        nc.vector.reciprocal(ilm1[:], lm1[:])
                        v = ppool.tile([P, WPP], F32, tag="v")
                        nc.vector.tensor_tensor(v[:], rng[:], ilm1[:], AL.mult)

                        # ---- big elementwise passes (in-place on F) ----
                        def bcast(t, nch):
                            return (
                                t[:]
                                .rearrange("p (o w) -> p o w", o=1)
                                .to_broadcast((P, nch, WPP))
                            )

                        def halves():
                            res = []
                            if DVE_CH:
                                res.append((nc.vector, slice(0, DVE_CH)))
                            if DVE_CH < C:
                                res.append((nc.gpsimd, slice(DVE_CH, C)))
                            return res

                        for eng, csl in halves():
                            nch = csl.stop - csl.start
                            Fs = Fv[:, csl, :]
                            eng.tensor_tensor(Fs, Fs, bcast(u, nch), AL.mult)
                            eng.tensor_tensor(Fs, Fs, bcast(c2, nch), AL.add)
                        # rounding: fp32 +M (ACT, free engine; Copy keeps the
                        # bias as an immediate instead of a const AP)
                        nc.scalar.activation(
                            F[:],
                            F[:],
                            mybir.ActivationFunctionType.Copy,
                            bias=M_MAGIC,
                            scale=1.0,
                        )
                        for eng, csl in halves():
                            nch = csl.stop - csl.start
                            Fs = Fv[:, csl, :]
                            eng.scalar_tensor_tensor(
                                Fs, Fs, M_MAGIC, bcast(v, nch), AL.subtract, AL.mult
                            )
                            eng.tensor_tensor(Fs, Fs, bcast(fmin, nch), AL.add)

                        # ---- store ----
                        for cc in range(0, C, CCH):
                            dst = out[b, cc : cc + CCH, px0 : px0 + SB_PX]
                            dst = dst.rearrange("c (p w) -> p c w", p=P)
                            nc.sync.dma_start(out=dst, in_=Fv[:, cc : cc + CCH, :])
    _split_sync_waits(nc)
    return nc


_nc_cache: dict[int, bass.Bass] = {}


def _get_nc(reps: int = 1) -> bass.Bass:
    if reps not in _nc_cache:
        _nc_cache[reps] = build(reps)
    return _nc_cache[reps]


def _in_maps(features: np.ndarray, bit_allocation: np.ndarray):
    f = np.ascontiguousarray(features, dtype=np.float32).reshape(B_FULL, C, PX)
    ba = np.ascontiguousarray(bit_allocation, dtype=np.int32).reshape(B_FULL, PX)
    maps = []
    for i in range(N_CORES):
        b0 = i * B_LOC
        maps.append(
            {
                "features": f[b0 : b0 + B_LOC],
                "bit_allocation": ba[b0 : b0 + B_LOC],
            }
        )
    return maps


def run(features: np.ndarray, bit_allocation: np.ndarray, reps: int = 1):
    nc = _get_nc(reps)
    maps = _in_maps(features, bit_allocation)
    res = run_bass_kernel_spmd(nc, maps, core_ids=list(range(N_CORES)))
    outs = [res.results[i]["out"].reshape(B_LOC, C, H, W) for i in range(N_CORES)]
    return np.concatenate(outs, axis=0)


def kernel(features: np.ndarray, bit_allocation: np.ndarray) -> np.ndarray:
    return run(features, bit_allocation, reps=1)


# revision 33
# speedup vs baseline: 208.7644x; 208.7644x over previous
"""AdaptiveQuantizer Trainium2 kernel (8 NeuronCores, data-parallel over batch).

Math (per pixel (b,h,w), over C=64 channels):
    fmin/fmax = min/max over channels
    rng  = fmax - fmin                (clamped to >= 1e-30; rng ~ 5 for randn data)
    lm1  = 2**bits - 1                (exact, via int shift trick)
    u    = lm1 / rng
    v    = rng / lm1
    w    = u*f - u*fmin               in [0, lm1]
    r    = round_half_even(w)         via fp32 add of M = 1.5*2**23
    out  = v*r + fmin

Layout: pixels in both the partition dim (128) and the free dim (256/superblock),
channels are a strided free-axis dim of one big SBUF tile -> channel reduction is
a free-axis strided tensor_reduce; per-pixel params are [128,256] tiles applied
with broadcast (step-0) APs. All elementwise work runs in-place on the big tile.
"""

import os
import sys
from contextlib import nullcontext

for _p in ("/opt/trn_rl_repo", "/root/.axon_site/_ro/trn_rl_repo"):
    if os.path.isdir(_p) and _p not in sys.path:
        sys.path.insert(0, _p)

import numpy as np

import concourse.bass as bass
import concourse.mybir as mybir
from concourse.bass_utils import run_bass_kernel_spmd
from concourse.tile import TileContext
from concourse.vector_clock import ScopedClock

# Problem shapes (hardcoded per spec)
B_FULL, C, H, W = 16, 64, 256, 256
N_CORES = 8
B_LOC = B_FULL // N_CORES  # images per core
PX = H * W                 # pixels per image
P = 128                    # SBUF partitions
WPP = 128                  # pixels per partition per superblock (default)
F_BUFS = 3                 # F-tile slots / pipeline depth (default)
CCH = 8                    # channels per DMA chunk (1 MiB per dma_start)
M_MAGIC = 12582912.0       # 1.5*2**23: fp32 "+M" add == round-to-nearest-even
AL = mybir.AluOpType
F32 = mybir.dt.float32
I32 = mybir.dt.int32

# All heavy passes on DVE: GPSIMD shares DVE's SBUF port (concurrent DVE+GP
# measured ~serial), Pool lacks min/max/ts/stt in this walrus anyway.
DVE_CH = 64
# "strided": one big tensor_reduce per stat. Chunked tt-accumulation loses:
# DVE DRAIN overhead punishes op count (big ops amortize it).
RED_IMPL = "strided"
# bf16 tail: q-stt writes bf16, final +fmin add runs in bf16 (2x mode), and
# the output DMA upcasts bf16->f32 via SWDGE (halves SBUF-port traffic).
BF16_TAIL = True
BF16 = mybir.dt.bfloat16

_drain_patched = False


def _patch_tile_drain():
    """This container's walrus accepts only ONE sync wait per TPB_CTRL
    instruction; Tile's final drain carries one wait per ticked proc.
    Split them across multiple drains."""
    global _drain_patched
    if _drain_patched:
        return
    _drain_patched = True

    def _patched(self, tick_clock, wait_clock):
        nc = self.nc
        drain_inst = nc.sync.drain()
        wait_clock.add_sem_waits(
            drain_inst.ins, ScopedClock({None: tick_clock.global_clock})
        )
        si = drain_inst.ins.sync_info
        waits = list(si.on_wait) if (si is not None and si.on_wait) else []
        if len(waits) > 1:
            si.on_wait = waits[:1]
            for wchunk in waits[1:]:
                extra = nc.sync.drain()
                esi = extra.ins.sync_info
                if esi is None:
                    extra.ins.sync_info = mybir.SyncInfo(
                        on_wait=[wchunk], on_update=[]
                    )
                else:
                    esi.on_wait = [wchunk]
        nc.all_engine_barrier()
        assert self.sems is not None
        popped = nc._tile_sem_poison_stack.pop()
        assert popped is self._sem_poison
        nc.clear_and_free_semaphores(list(self.sems.allocated().values()))
        nc.all_engine_barrier()

    TileContext._drain_and_barrier = _patched


def _split_sync_waits(nc: bass.Bass, max_waits: int = 1) -> None:
    """This container's walrus rejects instructions carrying more than one
    sync wait. Hoist excess waits onto injected same-engine NOPs placed
    immediately before the instruction (engine program order makes this
    semantically identical)."""
    k = 0
    for bb in nc.main_func.blocks:
        insts = list(bb.instructions)
        out_list = []
        changed = False
        for inst in insts:
            si = inst.sync_info
            waits = list(si.on_wait) if (si is not None and si.on_wait) else []
            if len(waits) > max_waits:
                keep = waits[-max_waits:]
                hoist = waits[:-max_waits]
                for i in range(0, len(hoist), max_waits):
                    nop = mybir.InstNoOp(name=f"WSPL-{k}", ins=[], outs=[])
                    k += 1
                    nop.engine = inst.engine
                    nop.sync_info = mybir.SyncInfo(
                        on_wait=hoist[i : i + max_waits], on_update=[]
                    )
                    out_list.append(nop)
                si.on_wait = keep
                changed = True
            out_list.append(inst)
        if changed:
            bb.instructions.clear()
            for inst in out_list:
                bb.instructions.append(inst)


def _superblock(nc, fpool, ppool, feat, bits, out, b, s, do_red, do_elem,
                timed=False, wpp=WPP, f_bufs=F_BUFS):
    WPP_, SB_PX = wpp, P * wpp
    px0 = s * SB_PX
    F = fpool.tile([P, C * WPP_], F32, tag="F", bufs=f_bufs)
    Fv = F[:].rearrange("p (c w) -> p c w", c=C)
    # ---- load: 1 MiB chunks, contiguous 1 KiB runs ----
    for cc in range(0, C, CCH):
        src = feat[b, cc : cc + CCH, px0 : px0 + SB_PX]
        src = src.rearrange("c (p w) -> p c w", p=P)
        nc.sync.dma_start(out=Fv[:, cc : cc + CCH, :], in_=src)
    bt = ppool.tile([P, WPP_], I32, tag="bt")
    nc.sync.dma_start(
        out=bt[:],
        in_=bits[b, px0 : px0 + SB_PX].rearrange("(p w) -> p w", p=P),
    )

    # ---- channel min/max (DVE only; Pool lacks min/max tensor_tensor) ----
    fmax = ppool.tile([P, WPP_], F32, tag="fmax")
    fmin = ppool.tile([P, WPP_], F32, tag="fmin")
    Fr = F[:].rearrange("p (c w) -> p w c", c=C)
    if do_red and RED_IMPL == "strided":
        nc.vector.tensor_reduce(
            fmax[:], Fr, axis=mybir.AxisListType.X, op=AL.max
        )
        nc.vector.tensor_reduce(
            fmin[:], Fr, axis=mybir.AxisListType.X, op=AL.min
        )
    elif do_red:
        # chunked: accumulate CCH-wide group min/max with tensor_tensor,
        # then one small strided fold per stat
        ngrp = C // CCH
        grp = [Fv[:, g * CCH : (g + 1) * CCH, :] for g in range(ngrp)]
        accmax = ppool.tile([P, CCH * WPP], F32, tag="accmax")
        accmin = ppool.tile([P, CCH * WPP], F32, tag="accmin")
        amx = accmax[:].rearrange("p (c w) -> p c w", c=CCH)
        amn = accmin[:].rearrange("p (c w) -> p c w", c=CCH)
        nc.vector.tensor_tensor(amx, grp[0], grp[1], AL.max)
        nc.vector.tensor_tensor(amn, grp[0], grp[1], AL.min)
        for g in range(2, ngrp):
            nc.vector.tensor_tensor(amx, amx, grp[g], AL.max)
            nc.vector.tensor_tensor(amn, amn, grp[g], AL.min)
        nc.vector.tensor_reduce(
            fmax[:],
            accmax[:].rearrange("p (c w) -> p w c", c=CCH),
            axis=mybir.AxisListType.X,
            op=AL.max,
        )
        nc.vector.tensor_reduce(
            fmin[:],
            accmin[:].rearrange("p (c w) -> p w c", c=CCH),
            axis=mybir.AxisListType.X,
            op=AL.min,
        )
    elif do_elem:
        nc.vector.memset(fmax[:], 1.0)
        nc.vector.memset(fmin[:], 0.0)

    if not do_elem:
        # bisection variant: ship F back out untouched
        for cc in range(0, C, CCH):
            dst = out[b, cc : cc + CCH, px0 : px0 + SB_PX]
            dst = dst.rearrange("c (p w) -> p c w", p=P)
            nc.sync.dma_start(out=dst, in_=Fv[:, cc : cc + CCH, :])
        return

    # ---- per-pixel params ([P, WPP_] tiles) ----
    # Reciprocals run on the otherwise-idle ScalarE as exp(-ln(x)) (Ln and
    # Exp share one ACT table set; ACT Reciprocal itself is banned).
    ACTF = mybir.ActivationFunctionType
    rng = ppool.tile([P, WPP_], F32, tag="rng")
    nc.vector.tensor_tensor(rng[:], fmax[:], fmin[:], AL.subtract)
    lnr = ppool.tile([P, WPP_], F32, tag="lnr")
    # bias acts as the div-by-zero guard: rng >= 0 always
    nc.scalar.activation(lnr[:], rng[:], ACTF.Ln, bias=1e-30, scale=1.0)
    rinv = ppool.tile([P, WPP_], F32, tag="rinv")
    nc.scalar.activation(rinv[:], lnr[:], ACTF.Exp, bias=0.0, scale=-1.0)
    # lm1 = 2**bits - 1 exactly: (bits+127)<<23 bitcast f32
    lvl_i = ppool.tile([P, WPP_], I32, tag="lvl_i")
    nc.vector.tensor_scalar_add(lvl_i[:], bt[:], 127)
    nc.vector.tensor_scalar(lvl_i[:], lvl_i[:], 23, None, AL.logical_shift_left)
    lm1 = ppool.tile([P, WPP_], F32, tag="lm1")
    nc.scalar.activation(lm1[:], lvl_i[:].bitcast(F32), ACTF.Copy, bias=-1.0, scale=1.0)
    u = ppool.tile([P, WPP_], F32, tag="u")
    nc.vector.tensor_tensor(u[:], lm1[:], rinv[:], AL.mult)
    # c2 = -u*fmin  (keep M out: c2 must stay sub-ulp accurate)
    c2 = ppool.tile([P, WPP_], F32, tag="c2")
    nc.vector.scalar_tensor_tensor(c2[:], u[:], -1.0, fmin[:], AL.mult, AL.mult)
    lnl = ppool.tile([P, WPP_], F32, tag="lnl")
    nc.scalar.activation(lnl[:], lm1[:], ACTF.Ln, bias=0.0, scale=1.0)
    ilm1 = ppool.tile([P, WPP_], F32, tag="ilm1")
    nc.scalar.activation(ilm1[:], lnl[:], ACTF.Exp, bias=0.0, scale=-1.0)
    v = ppool.tile([P, WPP_], F32, tag="v")
    nc.vector.tensor_tensor(v[:], rng[:], ilm1[:], AL.mult)

    # ---- big elementwise passes (in-place on F) ----
    def bcast(t, nch):
        return t[:].rearrange("p (o w) -> p o w", o=1).to_broadcast((P, nch, WPP_))

    def halves():
        res = []
        if DVE_CH:
            res.append((nc.vector, slice(0, DVE_CH)))
        if DVE_CH < C:
            res.append((nc.gpsimd, slice(DVE_CH, C)))
        return res

    for eng, csl in halves():
        nch = csl.stop - csl.start
        Fs = Fv[:, csl, :]
        eng.tensor_tensor(Fs, Fs, bcast(u, nch), AL.mult)
        eng.tensor_tensor(Fs, Fs, bcast(c2, nch), AL.add)
    # rounding: fp32 +M (ACT, free engine; Copy keeps bias as an immediate)
    nc.scalar.activation(
        F[:], F[:], mybir.ActivationFunctionType.Copy, bias=M_MAGIC, scale=1.0
    )
    if BF16_TAIL:
        # q = (s - M) * v, cast to bf16 on write; final +fmin in bf16 (2x);
        # output DMA upcasts bf16 -> f32 (SWDGE)
        Fb = fpool.tile(
            [P, C * WPP_], BF16, tag="Fb",
            bufs=int(os.environ.get("KFBBUFS", "1")),
        )
        Fbv = Fb[:].rearrange("p (c w) -> p c w", c=C)
        nc.vector.scalar_tensor_tensor(
            Fbv, Fv, M_MAGIC, bcast(v, C), AL.subtract, AL.mult
        )
        fmin_b = ppool.tile([P, WPP_], BF16, tag="fmin_b")
        nc.scalar.activation(
            fmin_b[:], fmin[:], mybir.ActivationFunctionType.Copy, bias=0.0, scale=1.0
        )
        fb_bc = (
            fmin_b[:]
            .rearrange("p (o w) -> p o w", o=1)
            .to_broadcast((P, C, WPP_))
        )
        nc.vector.tensor_tensor(Fbv, Fbv, fb_bc, AL.add)
        for cc in range(0, C, CCH):
            dst = out[b, cc : cc + CCH, px0 : px0 + SB_PX]
            dst = dst.rearrange("c (p w) -> p c w", p=P)
            if timed:
                # SWDGE inside For_i miscompiles ("ISA wrong length");
                # proxy with HWDGE bf16->bf16 (out is declared bf16)
                nc.sync.dma_start(out=dst, in_=Fbv[:, cc : cc + CCH, :])
            else:
                nc.gpsimd.dma_start(out=dst, in_=Fbv[:, cc : cc + CCH, :])
        return

    # q = (s - M) * v : fused scalar_tensor_tensor, DVE-only (Pool lacks it)
    nc.vector.scalar_tensor_tensor(
        Fv, Fv, M_MAGIC, bcast(v, C), AL.subtract, AL.mult
    )
    for eng, csl in halves():
        nch = csl.stop - csl.start
        Fs = Fv[:, csl, :]
        eng.tensor_tensor(Fs, Fs, bcast(fmin, nch), AL.add)

    # ---- store ----
    for cc in range(0, C, CCH):
        dst = out[b, cc : cc + CCH, px0 : px0 + SB_PX]
        dst = dst.rearrange("c (p w) -> p c w", p=P)
        nc.sync.dma_start(out=dst, in_=Fv[:, cc : cc + CCH, :])


def build(
    reps: int = 1,
    variant: str = "full",
    timed_loop: int = 0,
    wpp: int = None,
    f_bufs: int = None,
) -> bass.Bass:
    """Build the per-core Bass program.

    reps: python-unrolled repetitions of the whole (idempotent) workload.
    variant: full | dma | dma_red | dma_elem (bisection aids).
    timed_loop: if >0, build a timing-only program: internal DRAM tensors
    (no input upload), tiny dummy output, and a hardware For_i loop running
    the workload `timed_loop` times.
    """
    _patch_tile_drain()
    if wpp is None:
        wpp = int(os.environ.get("KWPP", WPP))
    if f_bufs is None:
        f_bufs = int(os.environ.get("KFBUFS", F_BUFS))
    n_sb = PX // (P * wpp)
    do_red = variant in ("full", "dma_red")
    do_elem = variant in ("full", "dma_elem")
    nc = bass.Bass()
    # register the Ln-bias epsilon as a const AP (non-Copy activation biases
    # must be const APs in this bass version)
    _eps_t = nc.alloc_sbuf_tensor("const-float32-epsln", [128, 1], F32)
    nc.gpsimd.memset(_eps_t.ap(), 1e-30)
    nc.const_aps.aps[(F32, 1e-30)] = _eps_t.ap()
    nc.all_engine_barrier()
    if timed_loop:
        feat = nc.dram_tensor("features_i", [B_LOC, C, PX], F32)
        bits = nc.dram_tensor("bits_i", [B_LOC, PX], I32)
        out_dt = BF16 if (BF16_TAIL and variant in ("full", "dma_elem")) else F32
        out = nc.dram_tensor("out_i", [B_LOC, C, PX], out_dt)
        dummy = nc.declare_dram_parameter("out", [1, 128], F32, isOutput=True)
    else:
        feat = nc.declare_dram_parameter(
            "features", [B_LOC, C, PX], F32, isOutput=False
        )
        bits = nc.declare_dram_parameter(
            "bit_allocation", [B_LOC, PX], I32, isOutput=False
        )
        out = nc.declare_dram_parameter("out", [B_LOC, C, PX], F32, isOutput=True)

    with TileContext(nc) as tc:
        with (
            tc.tile_pool(name="fpool", bufs=2) as fpool,
            tc.tile_pool(name="ppool", bufs=2) as ppool,
        ):
            loop_cm = tc.For_i(0, timed_loop, 1) if timed_loop else nullcontext()
            with loop_cm:
                for _rep in range(reps):
                    for b in range(B_LOC):
                        for s in range(n_sb):
                            _superblock(
                                nc, fpool, ppool, feat, bits, out, b, s,
                                do_red, do_elem, timed=bool(timed_loop),
                                wpp=wpp, f_bufs=f_bufs,
                            )
            if timed_loop:
                dtile = ppool.tile([1, 128], F32, tag="dummy")
                nc.vector.memset(dtile[:], 0.0)
                nc.sync.dma_start(out=dummy[:], in_=dtile[:])
    _split_sync_waits(nc)
    return nc


_nc_cache: dict = {}


def _get_nc(reps: int = 1, variant: str = "full", timed_loop: int = 0) -> bass.Bass:
    key = (reps, variant, timed_loop)
    if key not in _nc_cache:
        _nc_cache[key] = build(reps, variant, timed_loop)
    return _nc_cache[key]


def _in_maps(features: np.ndarray, bit_allocation: np.ndarray):
    f = np.ascontiguousarray(features, dtype=np.float32).reshape(B_FULL, C, PX)
    ba = np.ascontiguousarray(bit_allocation, dtype=np.int32).reshape(B_FULL, PX)
    maps = []
    for i in range(N_CORES):
        b0 = i * B_LOC
        maps.append(
            {
                "features": f[b0 : b0 + B_LOC],
                "bit_allocation": ba[b0 : b0 + B_LOC],
            }
        )
    return maps


def run(
    features: np.ndarray,
    bit_allocation: np.ndarray,
    reps: int = 1,
    variant: str = "full",
):
    nc = _get_nc(reps, variant)
    maps = _in_maps(features, bit_allocation)
    res = run_bass_kernel_spmd(nc, maps, core_ids=list(range(N_CORES)))
    outs = [res.results[i]["out"].reshape(B_LOC, C, H, W) for i in range(N_CORES)]
    return np.concatenate(outs, axis=0)


def run_timed(timed_loop: int, variant: str = "full"):
    """Run the timing-only program (no input upload); returns nothing useful."""
    nc = _get_nc(1, variant, timed_loop)
    maps = [{} for _ in range(N_CORES)]
    run_bass_kernel_spmd(nc, maps, core_ids=list(range(N_CORES)))


def kernel(features: np.ndarray, bit_allocation: np.ndarray) -> np.ndarray:
    return run(features, bit_allocation, reps=1)
